# revision 30
# baseline (speedup 1.0000x reference)
"""Trainium2 Bass kernel for nn_EntityResolution (2-layer hetero GNN mean-agg).

Live computation (dead code in the reference eliminated):
    u      = concat(user_emb[user_nodes], user_features)            [NU, 96]
    Wh0    = u @ Wv0 + bv0                                          [NU, 64]
    h_web  = segment_mean(Wh0[visits_src], visits_dst, NW)          [NW, 64]
    g      = leaky_relu(h_web)
    h_user = segment_mean(g[vb_src], vb_dst, NU) @ Wb1 + bb1*[deg>0]
    (the Linear commutes past the mean; bias only where cnt>0)

Strategy (8 NeuronCores, SPMD single NEFF):
  - Layer-0 aggregation is dst-sharded; the host expands u rows per visit
    edge into degree-sorted, 1/deg-prescaled slot columns (bf16).  The
    device runs weights-stationary matmuls that ACCUMULATE a website's k
    slot-columns in PSUM (start/stop flags), applies leaky-relu straight
    out of PSUM, transposes to row-major and stores g rows PAIRED:
    agin[q] = [g[2q] ; g[2q+1]] as one 128-wide bf16 row.
  - One bf16 AllGather replicates the paired g table (agout [VP, 128]).
  - Layer-1 gathers g rows per vb edge with transpose-mode dma_gather
    straight from agout: idx = global row pair (< 32768, so a single int16
    index space), 256B per descriptor; even rows land on SBUF partitions
    0:64, odd rows on 64:128.  Users are sorted per-core by (even-deg,
    odd-deg); shared per-128-block slot profiles (max over cores) give a
    single call/reduce structure; per-core pads point at an all-zero pair.
    DVE segment-reduces merged same-k runs into a [128, strip] accumulator
    (even sums on partitions 0:64, odd on 64:128), multiplies by 1/deg,
    and three accumulated matmuls (even half, odd half via a duplicated
    W row-block, bias x mask) produce yT [64, USH] per 512-col slice.
    The host undoes the per-core user sort.
"""

import sys

for _p in ("/opt/trn_rl_repo",):
    if _p not in sys.path:
        sys.path.insert(0, _p)

import numpy as np
import ml_dtypes

NU, NW, E = 200000, 50000, 1000000
H = 64
NCORES = 8
USH_REAL, WSH_REAL = 25000, 6250
G1 = 512                               # phase-1 node group width
R1 = 6656                              # padded table rows per core (13*512)
VR = NCORES * R1                       # 53248 rows -> 26624 pairs
VP = VR // 2
USH = 25088                            # padded users per core (196*128)
NBLK = USH // 128                      # 196 layer-1 blocks
STRIP = 2048                           # acc strip width (user positions)
CALL = 512                             # max idxs per dma_gather call
NQ = 4

_cache = {}


def _wrap_idx(flat):
    """[N] slot-order indices -> [128, N//16] int16 (16-wrap, 8x replicate)."""
    assert len(flat) % 16 == 0
    assert flat.max() < 32768 and flat.min() >= 0, (flat.min(), flat.max())
    w = flat.reshape(-1, 16).T
    return np.tile(w, (8, 1)).astype(np.int16)


def _prepare(inputs):
    user_nodes = np.asarray(inputs["user_nodes"])
    user_features = np.asarray(inputs["user_features"], dtype=np.float32)
    user_emb = np.asarray(inputs["user_emb"], dtype=np.float32)
    Wv0 = np.asarray(inputs["Wv0"], dtype=np.float32)
    bv0 = np.asarray(inputs["bv0"], dtype=np.float32)
    Wb1 = np.asarray(inputs["Wb1"], dtype=np.float32)
    bb1 = np.asarray(inputs["bb1"], dtype=np.float32)
    vsrc = np.asarray(inputs["visits_src"]).astype(np.int64)
    vdst = np.asarray(inputs["visits_dst"]).astype(np.int64)
    bsrc = np.asarray(inputs["vb_src"]).astype(np.int64)
    bdst = np.asarray(inputs["vb_dst"]).astype(np.int64)

    u97 = np.concatenate(
        [user_emb[user_nodes], user_features, np.ones((NU, 1), np.float32)],
        axis=1)
    W97 = np.concatenate([Wv0, bv0[None, :]], axis=0).astype(ml_dtypes.bfloat16)

    # ---- phase 1: per-core degree-sorted website packing, shared k profile --
    deg_w = np.bincount(vdst, minlength=NW).astype(np.int64)
    rec_w = 1.0 / np.maximum(deg_w, 1.0).astype(np.float32)
    worder = []                       # per core: sorted website ids (local)
    wdeg_s = []                       # per core: sorted degs, padded to R1
    for c in range(NCORES):
        dl = deg_w[c * WSH_REAL:(c + 1) * WSH_REAL]
        o = np.argsort(-dl, kind="stable")
        worder.append(o)
        wdeg_s.append(np.concatenate([dl[o], np.zeros(R1 - WSH_REAL, np.int64)]))
    prof1 = np.max(wdeg_s, 0)
    kg = prof1.reshape(-1, G1).max(1)            # [13] group k profile
    slots1 = int((kg * G1).sum())
    gbase = np.concatenate([[0], np.cumsum(kg * G1)])[:-1]

    # global packed row of each website
    grow = np.empty(NW, np.int64)
    for c in range(NCORES):
        grow[c * WSH_REAL + worder[c]] = c * R1 + np.arange(WSH_REAL)

    # per-core CSR of visit edges grouped by website
    ord_v = np.argsort(vdst, kind="stable")
    vs_by_w = vsrc[ord_v]                         # srcs grouped by website id
    wptr = np.concatenate([[0], np.cumsum(deg_w)])

    uTs_list = []
    for c in range(NCORES):
        cols = np.zeros((97, slots1), dtype=np.float32)
        dsorted = wdeg_s[c][:WSH_REAL]
        wid = c * WSH_REAL + worder[c]            # website at sorted pos
        for g in range(len(kg)):
            k = int(kg[g])
            lo, hi = g * G1, min((g + 1) * G1, WSH_REAL)
            if lo >= hi:
                continue
            n = hi - lo
            d = dsorted[lo:hi]
            jj = np.arange(k)[None, :]
            valid = jj < d[:, None]
            gath = wptr[wid[lo:hi]][:, None] + jj
            srcs = np.where(valid, vs_by_w[np.minimum(gath, len(vs_by_w) - 1)], 0)
            vals = u97[srcs.reshape(-1)].reshape(n, k, 97)
            vals = vals * (valid[:, :, None] * rec_w[wid[lo:hi]][:, None, None])
            # column = gbase[g] + j*512 + node
            cols[:, gbase[g] + jj[0][:, None] * G1 + np.arange(n)[None, :]] = \
                vals.transpose(2, 1, 0)
        uTs_list.append(cols.astype(ml_dtypes.bfloat16))

    # zero pair per core-shard for pads (rows 6250..6655 are zero)
    wz = 0 * R1 // 2 + (WSH_REAL // 2 + 1)        # pair 3126 of core 0: rows 6252/3

    # ---- layer 1: per-core user sort, shared block profiles ----------------
    ord_b = np.argsort(bdst, kind="stable")
    bs_by_u = bsrc[ord_b]
    deg_u = np.bincount(bdst, minlength=NU).astype(np.int64)
    uptr = np.concatenate([[0], np.cumsum(deg_u)])

    ke_all = np.zeros(NU, np.int64)
    ko_all = np.zeros(NU, np.int64)
    gr = grow[bsrc]
    np.add.at(ke_all, bdst[gr % 2 == 0], 1)
    np.add.at(ko_all, bdst[gr % 2 == 1], 1)
    grb = grow[bs_by_u]                           # rows in dst-sorted edge order

    uorder = []                                   # per core sorted user ids (local)
    keS, koS = [], []
    for c in range(NCORES):
        ke = ke_all[c * USH_REAL:(c + 1) * USH_REAL]
        ko = ko_all[c * USH_REAL:(c + 1) * USH_REAL]
        o = np.lexsort((-ko, -ke))
        uorder.append(o)
        pad = USH - USH_REAL
        keS.append(np.concatenate([ke[o], np.zeros(pad, np.int64)]))
        koS.append(np.concatenate([ko[o], np.zeros(pad, np.int64)]))
    Eprof = np.max([a.reshape(-1, 128).max(1) for a in keS], 0)   # [196]
    Oprof = np.max([a.reshape(-1, 128).max(1) for a in koS], 0)

    # ---- build shared call/reduce structure + per-core slot values ---------
    # Non-transpose gathers with the baseline-proven 128-wrap: a call covers
    # whole 128-user blocks; slot (g, j, p) sits at idx g*k*128 + j*128 + p
    # and lands at gt[p, g*k + j, :] (a 128-elem pair row).  Blocks of k<=4
    # with equal k share calls (g = 4//k blocks); k>4 blocks split into <=4
    # column chunks that accumulate into the strip acc.
    nstrip = (USH + STRIP - 1) // STRIP
    SB = STRIP // 128                     # blocks per strip
    calls = []      # n_idx per call
    reduces = []    # (cid, col0, g, k, cls, blk0, mode) mode: 0=write 1=add
    segs = []       # (cid, col0, blk, j0, jw, cls) for idx fill
    for s in range(nstrip):
        b0, b1 = s * SB, min((s + 1) * SB, NBLK)
        for cls, prof in ((0, Eprof), (1, Oprof)):
            b = b0
            while b < b1:
                k = int(prof[b])
                if k == 0:
                    b += 1
                    continue
                if k <= 4:
                    gpc = 4 // k
                    e = b
                    while (e < b1 and prof[e] == k and e - b < gpc):
                        e += 1
                    gg = e - b
                    cid = len(calls)
                    calls.append(gg * k * 128)
                    reduces.append((cid, 0, gg, k, cls, b, 0))
                    for gi in range(gg):
                        segs.append((cid, gi * k, b + gi, 0, k, cls))
                    b = e
                else:
                    j0 = 0
                    while j0 < k:
                        jw = min(4, k - j0)
                        cid = len(calls)
                        calls.append(jw * 128)
                        reduces.append((cid, 0, 1, jw, cls, b,
                                        0 if j0 == 0 else 1))
                        segs.append((cid, 0, b, j0, jw, cls))
                        j0 += jw
                    b += 1
    call_n = calls
    call_off = np.concatenate([[0], np.cumsum(call_n)]).astype(np.int64)
    tot_idx = int(call_off[-1])
    if tot_idx % 16:
        tot_idx += 16 - tot_idx % 16

    # strip boundaries in call ids
    strip_first_call = [None] * nstrip
    strip_last_call = [0] * nstrip
    for (cid, c0, g, k, cls, blk0, mode) in reduces:
        s = blk0 // SB
        if strip_first_call[s] is None:
            strip_first_call[s] = cid
        strip_last_call[s] = max(strip_last_call[s], cid)

    # per-core idx arrays: per-user class edge lists, then scatter to slots
    idx_list = []
    for c in range(NCORES):
        keS_c, koS_c = keS[c], koS[c]
        uid = np.full(USH, -1, np.int64)
        uid[:USH_REAL] = c * USH_REAL + uorder[c]
        # edge pair lists per position, per class, padded to profile k
        maxk = [int(Eprof.max()), int(Oprof.max())]
        rowsP = []
        for cls in (0, 1):
            K = maxk[cls]
            out = np.full((USH, K), wz, np.int64)
            act = uid >= 0
            base = uptr[uid[act]]
            degs = deg_u[uid[act]]
            maxd = int(degs.max()) if len(degs) else 0
            if maxd:
                ii = np.arange(maxd)[None, :]
                ev = ii < degs[:, None]
                allr = np.where(ev, grb[np.minimum(base[:, None] + ii,
                                                   len(grb) - 1)], -1)
                want = (allr >= 0) & ((allr % 2) == cls)
                cnt = want.sum(1)
                idxm = np.where(want, allr // 2, 0)
                ordr = np.argsort(~want, axis=1, kind="stable")
                comp = np.take_along_axis(idxm, ordr, 1)
                if comp.shape[1] < K:
                    comp = np.pad(comp, ((0, 0), (0, K - comp.shape[1])))
                comp = comp[:, :K]
                vmask = np.arange(K)[None, :] < np.minimum(cnt, K)[:, None]
                out[act] = np.where(vmask, comp, wz)
            rowsP.append(out)
        flat = np.full(tot_idx, wz, np.int64)
        for (cid, col0, blk, j0, jw, cls) in segs:
            pos = blk * 128 + np.arange(128)
            vals = rowsP[cls][pos, j0:j0 + jw]          # [128, jw]
            o = call_off[cid] + col0 * 128
            flat[o:o + jw * 128] = vals.T.reshape(-1)   # (j, p) order
        idx_list.append(_wrap_idx(flat))

    # recip / mask in sorted order, replicated layouts
    recip_list, mask_list, inv_list = [], [], []
    for c in range(NCORES):
        du = deg_u[c * USH_REAL:(c + 1) * USH_REAL].astype(np.float32)
        r = np.zeros(USH, np.float32)
        r[:USH_REAL] = (1.0 / np.maximum(du, 1.0))[uorder[c]]
        mk = np.zeros(USH, np.float32)
        mk[:USH_REAL] = (du > 0).astype(np.float32)[uorder[c]]
        recip_list.append(r.reshape(NBLK, 128).T.copy())   # [128, NBLK]
        mask_list.append(mk[None, :].astype(ml_dtypes.bfloat16))
        inv = np.empty(USH_REAL, np.int64)
        inv[uorder[c]] = np.arange(USH_REAL)
        inv_list.append(inv)

    W65 = np.concatenate([Wb1, bb1[None, :]], axis=0).astype(ml_dtypes.bfloat16)

    static = dict(kg=[int(x) for x in kg], gbase=[int(x) for x in gbase],
                  slots1=slots1, call_n=call_n,
                  call_off=[int(x) for x in call_off], reduces=reduces,
                  strip_first_call=strip_first_call,
                  strip_last_call=strip_last_call,
                  nstrip=nstrip, tot_idx=tot_idx)
    percore = []
    for c in range(NCORES):
        percore.append({
            "uTs": uTs_list[c], "W97": W97, "W65": W65,
            "idx1": idx_list[c], "recip": recip_list[c].astype(np.float32),
            "mask": mask_list[c],
        })
    return static, percore, inv_list


def _build(static):
    import os
    import concourse.bacc as bacc
    import concourse.mybir as mybir
    import concourse.tile as tile
    from concourse import library_config
    from concourse.masks import make_identity

    PH = int(os.environ.get("K_PHASES", "9"))
    P3 = int(os.environ.get("K_P3", "9"))   # 1=gather 2=+reduce 3=+mult 4=+mm
    f32, bf16, i16 = mybir.dt.float32, mybir.dt.bfloat16, mybir.dt.int16
    AX = mybir.AxisListType.X

    kg, gbase, slots1 = static["kg"], static["gbase"], static["slots1"]
    call_n, call_off = static["call_n"], static["call_off"]
    reduces, nstrip = static["reduces"], static["nstrip"]
    sfc, slc = static["strip_first_call"], static["strip_last_call"]
    tot_idx = static["tot_idx"]

    nc = bacc.Bacc("TRN2", target_bir_lowering=False, debug=False,
                   num_devices=NCORES, num_swdge_queues=NQ)

    uTs = nc.dram_tensor("uTs", [97, slots1], bf16, kind="ExternalInput")
    W97 = nc.dram_tensor("W97", [97, H], bf16, kind="ExternalInput")
    W65 = nc.dram_tensor("W65", [65, H], bf16, kind="ExternalInput")
    idx1 = nc.dram_tensor("idx1", [128, tot_idx // 16], i16,
                          kind="ExternalInput")
    recip = nc.dram_tensor("recip", [128, NBLK], f32, kind="ExternalInput")
    mask = nc.dram_tensor("mask", [1, USH], bf16, kind="ExternalInput")
    yT = nc.dram_tensor("yT", [H, USH], f32, kind="ExternalOutput")

    agin = nc.dram_tensor("agin", [R1 // 2, 128], bf16)
    agout = nc.dram_tensor("agout", [VP, 128], bf16, addr_space="Shared")
    # +1 pad row: the odd-class view reads 64 elems past the last pair
    gtbl = nc.dram_tensor("gtbl", [VP + 1, 128], bf16)
    DBG = int(os.environ.get("K_DBG", "0"))
    if DBG:
        dbg = nc.dram_tensor("dbg", [VP, 128], bf16, kind="ExternalOutput")

    qn = [0]

    def nextq():
        qn[0] = (qn[0] + 1) % NQ
        return qn[0]

    ldq = [0]

    def ldeng():
        ldq[0] += 1
        return nc.sync if ldq[0] % 2 == 0 else nc.scalar

    LOADW = 4096

    with tile.TileContext(nc) as tc:
        nc.gpsimd.load_library(library_config.mlp)
        with (
            tc.tile_pool(name="const", bufs=1) as cpool,
            tc.tile_pool(name="stream", bufs=4) as spool,
            tc.tile_pool(name="gather",
                         bufs=int(os.environ.get("K_GB", "8"))) as gpool,
            tc.tile_pool(name="acc", bufs=2) as apool,
            tc.tile_pool(name="strip", bufs=2) as stpool,
            tc.tile_pool(name="small", bufs=4) as smpool,
            tc.tile_pool(name="ps1", bufs=3, space="PSUM") as ps1,
            tc.tile_pool(name="psT", bufs=1, space="PSUM") as psTp,
            tc.tile_pool(name="psy", bufs=2, space="PSUM") as psyp,
        ):
            W97_t = cpool.tile([97, H], bf16, tag="w97")
            nc.sync.dma_start(W97_t[:], W97[:, :])
            W65_t = cpool.tile([65, H], bf16, tag="w65")
            nc.sync.dma_start(W65_t[:], W65[:, :])
            idx_t = cpool.tile([128, tot_idx // 16], i16, tag="idx")
            nc.scalar.dma_start(idx_t[:], idx1[:, :])
            rec_t = cpool.tile([128, NBLK], f32, tag="rec")
            nc.scalar.dma_start(rec_t[:], recip[:, :])
            mask_t = cpool.tile([1, USH], bf16, tag="mask")
            nc.scalar.dma_start(mask_t[:], mask[:, :])
            ident = cpool.tile([128, 128], bf16, tag="ident")
            make_identity(nc, ident[:])

            # ---- phase 1: accumulate k slot-columns per website in PSUM ----
            if PH >= 1:
                for g, k in enumerate(kg):
                    ps = ps1.tile([64, G1], f32, space="PSUM", tag="mm0")
                    ncols = k * G1
                    pos = 0
                    while pos < ncols:
                        ln = min(LOADW, ncols - pos)
                        st = spool.tile([97, LOADW], bf16, tag="uTs")
                        ldeng().dma_start(
                            st[:, :ln], uTs[:, gbase[g] + pos:gbase[g] + pos + ln])
                        for j in range(ln // G1):
                            jabs = (pos + j * G1) // G1
                            nc.tensor.matmul(
                                ps[:], lhsT=W97_t[:],
                                rhs=st[:, j * G1:(j + 1) * G1],
                                start=(jabs == 0), stop=(jabs == k - 1))
                        pos += ln
                    # leaky-relu straight out of PSUM, then pair-transpose
                    gTl = smpool.tile([64, G1], bf16, tag="gTl")
                    nc.scalar.activation(gTl[:], ps[:],
                                         mybir.ActivationFunctionType.Lrelu,
                                         alpha=0.01)
                    sb = smpool.tile([128, 2, 2, H], bf16, tag="sb")
                    for blk in range(2):
                        for half in range(2):
                            pt = psTp.tile([128, H], bf16, space="PSUM",
                                           tag="tr")
                            nc.tensor.transpose(
                                pt[:],
                                gTl[:, blk * 256 + half:blk * 256 + 256:2],
                                ident[:64, :64])
                            nc.vector.tensor_copy(sb[:, blk, half, :], pt[:])
                    nc.sync.dma_start(
                        agin[g * 256:(g + 1) * 256, :]
                        .rearrange("(b p) d -> p b d", p=128),
                        sb[:].rearrange("p b h d -> p b (h d)"))

            # ---- phase 2: allgather paired g rows (bf16) ----
            if PH >= 2:
                nc.gpsimd.collective_compute(
                    "AllGather", mybir.AluOpType.bypass,
                    ins=[agin[:, :]], outs=[agout[:, :]],
                    replica_groups=[list(range(NCORES))])

            # ---- phase 2b: stage the gathered table into local DRAM ----
            # (SWDGE gathers from the Shared scratchpad hang on HW; copy to a
            # core-local tensor first, split across both HWDGE queues.)
            if PH >= 3:
                NCP = 8
                step = (VP + NCP - 1) // NCP
                for i in range(NCP):
                    r0, r1 = i * step, min((i + 1) * step, VP)
                    eng = nc.sync if i % 2 == 0 else nc.scalar
                    eng.dma_start(gtbl[r0:r1, :], agout[r0:r1, :])
                if DBG:
                    nc.sync.dma_start(dbg[:, :], gtbl[:VP, :])

            # ---- phase 3: per-strip gather + reduce + transpose + matmul ----
            if PH >= 3:
                SB = STRIP // 128
                red_by_call = {}
                for r in reduces:
                    red_by_call.setdefault(r[0], []).append(r)
                for s in range(nstrip):
                    p0 = s * STRIP
                    b0 = s * SB
                    nb = min(SB, NBLK - b0)
                    accE = apool.tile([128, SB, H], f32, tag="accE")
                    accO = apool.tile([128, SB, H], f32, tag="accO")
                    nc.vector.memset(accE[:], 0.0)
                    nc.vector.memset(accO[:], 0.0)
                    if sfc[s] is not None:
                        for cid in range(sfc[s], slc[s] + 1):
                            n = call_n[cid]
                            gt = gpool.tile([128, 4, 128], bf16, tag="gt")
                            nc.gpsimd.dma_gather(
                                gt[:, :n // 128, :], gtbl[:VP, :],
                                idx_t[:, call_off[cid] // 16:
                                      (call_off[cid] + n) // 16],
                                n, n, 128, queue_num=nextq())
                            for (_, c0, g, k, cls, blk0, mode) in \
                                    red_by_call.get(cid, []):
                                if P3 < 2:
                                    break
                                acc = accE if cls == 0 else accO
                                ds = slice(0, 64) if cls == 0 else \
                                    slice(64, 128)
                                bo = blk0 - b0
                                if mode == 0:
                                    if k > 1:
                                        nc.vector.reduce_sum(
                                            acc[:, bo:bo + g, :],
                                            gt[:, c0:c0 + g * k, ds]
                                            .rearrange("p (g k) d -> p g d k",
                                                       k=k),
                                            axis=AX)
                                    else:
                                        nc.vector.tensor_copy(
                                            acc[:, bo:bo + g, :],
                                            gt[:, c0:c0 + g, ds])
                                else:
                                    tmp = smpool.tile([128, H], f32, tag="tmp")
                                    if k > 1:
                                        nc.vector.reduce_sum(
                                            tmp[:].rearrange(
                                                "p (g d) -> p g d", g=1),
                                            gt[:, c0:c0 + k, ds].rearrange(
                                                "p (g k) d -> p g d k", k=k),
                                            axis=AX)
                                    else:
                                        nc.vector.tensor_copy(
                                            tmp[:], gt[:, c0:c0 + 1, ds]
                                            .rearrange("p g d -> p (g d)"))
                                    nc.vector.tensor_add(
                                        acc[:, bo, :], acc[:, bo, :], tmp[:])
                    ob = stpool.tile([128, SB, H], bf16, tag="ob")
                    if P3 >= 3:
                        nc.vector.tensor_add(
                            accE[:].rearrange("p g d -> p (g d)"),
                            accE[:].rearrange("p g d -> p (g d)"),
                            accO[:].rearrange("p g d -> p (g d)"))
                        nc.vector.tensor_tensor(
                            out=ob[:, :nb, :], in0=accE[:, :nb, :],
                            in1=rec_t[:, b0:b0 + nb].to_broadcast(
                                [128, nb, H]),
                            op=mybir.AluOpType.mult)
                    else:
                        nc.vector.memset(ob[:].rearrange("p g d -> p (g d)"),
                                         0.0)
                    yb = stpool.tile([64, STRIP], f32, tag="yb")
                    if P3 >= 4:
                        for t0 in range(0, nb, 4):
                            tw = min(4, nb - t0)
                            psT = psTp.tile([64, 512], bf16, space="PSUM",
                                            tag="trT")
                            for t in range(tw):
                                nc.tensor.transpose(
                                    psT[:, t * 128:(t + 1) * 128],
                                    ob[:, t0 + t, :], ident[:, :128])
                            rhs = smpool.tile([65, 512], bf16, tag="rhs")
                            nc.vector.tensor_copy(rhs[0:64, :tw * 128],
                                                  psT[:, :tw * 128])
                            u0 = p0 + t0 * 128
                            nc.vector.tensor_copy(
                                rhs[64:65, :tw * 128],
                                mask_t[:, u0:u0 + tw * 128])
                            py = psyp.tile([64, 512], f32, space="PSUM",
                                           tag="mmy")
                            nc.tensor.matmul(py[:, :tw * 128], lhsT=W65_t[:],
                                             rhs=rhs[:, :tw * 128],
                                             start=True, stop=True)
                            nc.vector.tensor_copy(
                                yb[:, t0 * 128:t0 * 128 + tw * 128],
                                py[:, :tw * 128])
                    else:
                        nc.vector.memset(yb[:], 0.0)
                    pw = min(STRIP, USH - p0)
                    nc.sync.dma_start(yT[:, p0:p0 + pw], yb[:, :pw])

    nc.compile()
    return nc


def kernel(**inputs):
    from concourse.bass_utils import run_bass_kernel_spmd

    static, percore, inv_list = _prepare(inputs)
    if "nc" not in _cache:
        _cache["nc"] = _build(static)
    res = run_bass_kernel_spmd(_cache["nc"], percore,
                               core_ids=list(range(NCORES)))
    out = np.empty((NU, H), dtype=np.float32)
    for c in range(NCORES):
        out[c * USH_REAL:(c + 1) * USH_REAL] = \
            res.results[c]["yT"][:, inv_list[c]].T
    return out
